# revision 2
# baseline (speedup 1.0000x reference)
"""ANI-style per-species MLP (MoE routing) on 8 Trainium2 NeuronCores.

Strategy
--------
Data-parallel over molecules: core c gets molecules [512c, 512(c+1)).
Instead of the dense all-experts compute, atoms are sorted by species on
the host so each core runs only its own expert per segment (3.5x less
matmul work). Segments are padded to a common capacity CAP so all 8
cores run the same SPMD graph.

Device kernel (per core), feature-major layout:
  aevT [384, 4*CAP] bf16 in DRAM (host-transposed, species-sorted).
  Per 512-atom tile: L1/L2/L3/L4 matmuls (lhsT = weight chunks [K,M],
  rhs = activations [K,N<=512], PSUM f32), CELU between layers as
    celu(x+b) = max(x+b, min(0.1*e^{10(x+b)} - 0.1, 0))
  which maps to exactly three engine ops:
    ScalarE : g = Exp(10*x + (10b + ln 0.1))      PSUM -> SBUF bf16
    GpSimd  : t = (g - 0.1) min 0                 SBUF bf16 (4x-able op)
    VectorE : h = (x + b) max t                   PSUM+SBUF -> SBUF bf16
  L4 (96->1) packs 4 tiles' outputs into one PSUM bank via
  tile_position=(0,32j); one ScalarE bank copy + one strided DMA per
  group writes per-atom energies out.

Host post: unsort per-atom energies, add b4[species], sum per molecule.
bf16 end-to-end gives rel err ~1.4e-3 vs the f32 reference (gate 2e-2).
"""

import math
import sys
from contextlib import ExitStack

import numpy as np

try:
    import concourse.bass as bass
except ImportError:  # pragma: no cover
    sys.path.insert(0, "/opt/trn_rl_repo")
    import concourse.bass as bass

import ml_dtypes

import concourse.tile as tile
from concourse import mybir
from concourse.bass_utils import run_bass_kernel_spmd

BF16NP = ml_dtypes.bfloat16
F32 = mybir.dt.float32
BF16 = mybir.dt.bfloat16

NSPEC = 4
AEV = 384
DIMS = [384, 160, 128, 96, 1]
ALPHA = 0.1
NCORES = 8
LN_ALPHA = math.log(ALPHA)
INV_ALPHA = 1.0 / ALPHA


# --------------------------------------------------------------------------
# Workaround: the walrus build in this container rejects instructions whose
# sync_info carries more than a couple of semaphore waits ("Too many sync
# wait commands"). TileContext's tail drain can accumulate several. Move
# excess waits onto NoOps inserted before the offending instruction (the
# engine blocks on each in turn -> semantically identical).
_splitw_ctr = [0]


def _split_multi_waits(nc, maxw=1):
    for fn in nc.m.functions:
        for bb in fn.blocks:
            out = []
            changed = False
            for ins in bb.instructions:
                si = ins.sync_info
                if si is not None and si.on_wait is not None and len(si.on_wait) > maxw:
                    waits = list(si.on_wait)
                    overflow, keep = waits[:-maxw], waits[-maxw:]
                    for i in range(0, len(overflow), maxw):
                        _splitw_ctr[0] += 1
                        nop = mybir.InstNoOp(
                            name=f"bass_splitw_{_splitw_ctr[0]}", ins=[], outs=[]
                        )
                        nop.engine = ins.engine
                        nop.sync_info = mybir.SyncInfo(
                            on_wait=overflow[i : i + maxw], on_update=[]
                        )
                        nc.register_instruction(nop, overwrite=True)
                        out.append(nop)
                        changed = True
                    si.on_wait = keep
                out.append(ins)
            if changed:
                bb.instructions = out


def _tiles_for_cap(cap):
    tiles = []
    for s in range(NSPEC):
        off = 0
        while off < cap:
            n = min(512, cap - off)
            tiles.append((s, s * cap + off, n))
            off += n
    return tiles


def build_graph(cap, repeat=1):
    """Build the SPMD per-core graph. repeat>1 wraps the whole per-tile
    pipeline in a For_i loop (used only for on-device timing)."""
    nc = bass.Bass()
    rows = NSPEC * cap
    tiles = _tiles_for_cap(cap)
    nt = len(tiles)

    aevT = nc.declare_dram_parameter("aevT", [AEV, rows], BF16, isOutput=False)
    W1 = nc.declare_dram_parameter("W1", [NSPEC, 384, 160], BF16, isOutput=False)
    W2 = nc.declare_dram_parameter("W2", [NSPEC, 160, 128], BF16, isOutput=False)
    W3 = nc.declare_dram_parameter("W3", [NSPEC, 128, 96], BF16, isOutput=False)
    W4T = nc.declare_dram_parameter("W4T", [96, NSPEC], BF16, isOutput=False)
    EB1 = nc.declare_dram_parameter("EB1", [160, NSPEC], F32, isOutput=False)
    AB1 = nc.declare_dram_parameter("AB1", [160, NSPEC], F32, isOutput=False)
    EB2 = nc.declare_dram_parameter("EB2", [128, NSPEC], F32, isOutput=False)
    AB2 = nc.declare_dram_parameter("AB2", [128, NSPEC], F32, isOutput=False)
    EB3 = nc.declare_dram_parameter("EB3", [96, NSPEC], F32, isOutput=False)
    AB3 = nc.declare_dram_parameter("AB3", [96, NSPEC], F32, isOutput=False)
    EOUT = nc.declare_dram_parameter("eout", [nt, 512], F32, isOutput=True)

    Exp = mybir.ActivationFunctionType.Exp
    Copy = mybir.ActivationFunctionType.Copy
    SUB = mybir.AluOpType.subtract
    MIN = mybir.AluOpType.min
    ADD = mybir.AluOpType.add
    MAX = mybir.AluOpType.max

    with tile.TileContext(nc) as tc, ExitStack() as ctx:
        singles = ctx.enter_context(tc.tile_pool(name="singles", bufs=1))

        def load(name, src, p, f, dt):
            t = singles.tile([p, f], dt, tag=name)
            nc.sync.dma_start(t[:], src)
            return t

        w1 = {
            (s, k): load(f"w1_{s}_{k}", W1[s, 128 * k : 128 * (k + 1), :], 128, 160, BF16)
            for s in range(NSPEC)
            for k in range(3)
        }
        w2a = {s: load(f"w2a_{s}", W2[s, 0:128, :], 128, 128, BF16) for s in range(NSPEC)}
        w2b = {s: load(f"w2b_{s}", W2[s, 128:160, :], 32, 128, BF16) for s in range(NSPEC)}
        w3 = {s: load(f"w3_{s}", W3[s, :, :], 128, 96, BF16) for s in range(NSPEC)}
        w4 = load("w4", W4T[:, :], 96, NSPEC, BF16)
        eb1a = load("eb1a", EB1[0:128, :], 128, NSPEC, F32)
        eb1b = load("eb1b", EB1[128:160, :], 32, NSPEC, F32)
        ab1a = load("ab1a", AB1[0:128, :], 128, NSPEC, F32)
        ab1b = load("ab1b", AB1[128:160, :], 32, NSPEC, F32)
        eb2 = load("eb2", EB2[:, :], 128, NSPEC, F32)
        ab2 = load("ab2", AB2[:, :], 128, NSPEC, F32)
        eb3 = load("eb3", EB3[:, :], 96, NSPEC, F32)
        ab3 = load("ab3", AB3[:, :], 96, NSPEC, F32)

        aevp = ctx.enter_context(tc.tile_pool(name="aevp", bufs=9))
        gp = ctx.enter_context(tc.tile_pool(name="gp", bufs=3))
        tp = ctx.enter_context(tc.tile_pool(name="tp", bufs=3))
        hp = ctx.enter_context(tc.tile_pool(name="hp", bufs=3))
        ep = ctx.enter_context(tc.tile_pool(name="ep", bufs=2))
        p1a = ctx.enter_context(tc.tile_pool(name="p1a", bufs=2, space="PSUM"))
        p1b = ctx.enter_context(tc.tile_pool(name="p1b", bufs=2, space="PSUM"))
        p2 = ctx.enter_context(tc.tile_pool(name="p2", bufs=2, space="PSUM"))
        p3 = ctx.enter_context(tc.tile_pool(name="p3", bufs=1, space="PSUM"))
        p4 = ctx.enter_context(tc.tile_pool(name="p4", bufs=1, space="PSUM"))

        def celu(xp, p, n, eb, ab, kind):
            g = gp.tile([p, n], BF16, tag="g" + kind)
            nc.scalar.activation(out=g[:, :], in_=xp, func=Exp, bias=eb, scale=INV_ALPHA)
            t = tp.tile([p, n], BF16, tag="t" + kind)
            nc.gpsimd.tensor_scalar(
                out=t[:, :], in0=g[:, :], scalar1=ALPHA, scalar2=0.0, op0=SUB, op1=MIN
            )
            h = hp.tile([p, n], BF16, tag="h" + kind)
            nc.vector.scalar_tensor_tensor(
                out=h[:, :], in0=xp, scalar=ab, in1=t[:, :], op0=ADD, op1=MAX
            )
            return h

        def body():
            x4 = None
            gi0 = 0
            for idx, (s, col0, n) in enumerate(tiles):
                a = []
                for k in range(3):
                    at = aevp.tile([128, n], BF16, tag=f"aev{k}")
                    nc.sync.dma_start(
                        at[:, :], aevT[128 * k : 128 * (k + 1), col0 : col0 + n]
                    )
                    a.append(at)
                # L1: 384 -> 160 as M-chunks 128 + 32, K-chunks 3x128
                x1a = p1a.tile([128, 512], F32, tag="p1a")
                for k in range(3):
                    nc.tensor.matmul(
                        x1a[:, :n], w1[s, k][:, 0:128], a[k][:, :],
                        start=(k == 0), stop=(k == 2),
                    )
                x1b = p1b.tile([32, 512], F32, tag="p1b")
                for k in range(3):
                    nc.tensor.matmul(
                        x1b[:, :n], w1[s, k][:, 128:160], a[k][:, :],
                        start=(k == 0), stop=(k == 2),
                    )
                h1a = celu(x1a[:, :n], 128, n, eb1a[:, s : s + 1], ab1a[:, s : s + 1], "1a")
                h1b = celu(x1b[:, :n], 32, n, eb1b[:, s : s + 1], ab1b[:, s : s + 1], "1b")
                # L2: 160 -> 128, K-chunks 128 + 32
                x2 = p2.tile([128, 512], F32, tag="p2")
                nc.tensor.matmul(x2[:, :n], w2a[s][:, :], h1a[:, :], start=True, stop=False)
                nc.tensor.matmul(x2[:, :n], w2b[s][:, :], h1b[:, :], start=False, stop=True)
                h2 = celu(x2[:, :n], 128, n, eb2[:, s : s + 1], ab2[:, s : s + 1], "2")
                # L3: 128 -> 96
                x3 = p3.tile([96, 512], F32, tag="p3")
                nc.tensor.matmul(x3[:, :n], w3[s][:, :], h2[:, :], start=True, stop=True)
                h3 = celu(x3[:, :n], 96, n, eb3[:, s : s + 1], ab3[:, s : s + 1], "3")
                # L4: 96 -> 1; pack 4 tiles into one PSUM bank at partitions 0/32/64/96
                j = idx % 4
                if j == 0:
                    x4 = p4.tile([128, 512], F32, tag="p4")
                    gi0 = idx
                nc.tensor.matmul(
                    x4[32 * j : 32 * j + 1, :n], w4[:, s : s + 1], h3[:, :],
                    start=True, stop=True, tile_position=(0, 32 * j),
                )
                if j == 3 or idx == nt - 1:
                    gs = idx - gi0 + 1
                    esb = ep.tile([128, 512], F32, tag="esb")
                    nc.scalar.activation(out=esb[:, :], in_=x4[:, :], func=Copy)
                    nc.sync.dma_start(
                        EOUT[gi0 : gi0 + gs, :], esb[0 : 32 * gs : 32, :]
                    )

        if repeat > 1:
            with tc.For_i(0, repeat, 1):
                body()
        else:
            body()

    _split_multi_waits(nc)
    return nc


def prepare_inputs(species, aev, W1, b1, W2, b2, W3, b3, W4, b4, cap=None):
    """Host-side routing: per core, sort atoms by species, pad segments to
    a common capacity, transpose + cast aev. Returns (in_maps, meta)."""
    species = np.asarray(species)
    aev = np.asarray(aev, dtype=np.float32)
    B, A = species.shape
    bc = B // NCORES
    natoms = bc * A

    spf = species.reshape(NCORES, natoms)
    aevf = aev.reshape(NCORES, natoms, AEV)

    orders, counts = [], []
    for c in range(NCORES):
        orders.append(np.argsort(spf[c], kind="stable"))
        counts.append(np.bincount(spf[c].astype(np.int64), minlength=NSPEC))
    counts = np.stack(counts)
    if cap is None:
        cap = max(512, int(-(-counts.max() // 128) * 128))
    rows = NSPEC * cap

    # shared (replicated) weight-side tensors
    w1b = np.ascontiguousarray(W1.astype(BF16NP))
    w2b = np.ascontiguousarray(W2.astype(BF16NP))
    w3b = np.ascontiguousarray(W3.astype(BF16NP))
    w4t = np.ascontiguousarray(W4[:, :, 0].T.astype(BF16NP))  # [96, 4]
    shared = {
        "W1": w1b, "W2": w2b, "W3": w3b, "W4T": w4t,
        "EB1": np.ascontiguousarray((INV_ALPHA * b1 + LN_ALPHA).T.astype(np.float32)),
        "AB1": np.ascontiguousarray(b1.T.astype(np.float32)),
        "EB2": np.ascontiguousarray((INV_ALPHA * b2 + LN_ALPHA).T.astype(np.float32)),
        "AB2": np.ascontiguousarray(b2.T.astype(np.float32)),
        "EB3": np.ascontiguousarray((INV_ALPHA * b3 + LN_ALPHA).T.astype(np.float32)),
        "AB3": np.ascontiguousarray(b3.T.astype(np.float32)),
    }

    in_maps = []
    for c in range(NCORES):
        srt = aevf[c][orders[c]]  # [natoms, AEV] species-sorted
        padded = np.zeros((rows, AEV), dtype=BF16NP)
        off = 0
        for s in range(NSPEC):
            cnt = int(counts[c, s])
            padded[s * cap : s * cap + cnt] = srt[off : off + cnt]
            off += cnt
        aevT = np.ascontiguousarray(padded.T)  # [AEV, rows] bf16
        m = {"aevT": aevT}
        m.update(shared)
        in_maps.append(m)

    meta = {
        "cap": cap,
        "orders": orders,
        "counts": counts,
        "bc": bc,
        "A": A,
        "b4": np.asarray(b4, dtype=np.float32)[:, 0],
        "species": spf,
    }
    return in_maps, meta


def assemble_output(results, meta):
    cap = meta["cap"]
    bc, A = meta["bc"], meta["A"]
    tiles = _tiles_for_cap(cap)
    energies = np.empty((NCORES, bc), dtype=np.float32)
    for c in range(NCORES):
        eout = np.asarray(results[c]["eout"], dtype=np.float32)
        e_sorted = np.empty(NSPEC * cap, dtype=np.float32)
        for i, (s, col0, n) in enumerate(tiles):
            e_sorted[col0 : col0 + n] = eout[i, :n]
        e_atom = np.empty(bc * A, dtype=np.float32)
        off = 0
        order = meta["orders"][c]
        for s in range(NSPEC):
            cnt = int(meta["counts"][c, s])
            e_atom[order[off : off + cnt]] = e_sorted[s * cap : s * cap + cnt]
            off += cnt
        e_atom += meta["b4"][meta["species"][c].astype(np.int64)]
        energies[c] = e_atom.reshape(bc, A).sum(axis=1)
    return energies.reshape(-1)


_graph_cache = {}


def kernel(species, aev, W1, b1, W2, b2, W3, b3, W4, b4):
    in_maps, meta = prepare_inputs(species, aev, W1, b1, W2, b2, W3, b3, W4, b4)
    cap = meta["cap"]
    nc = _graph_cache.get(cap)
    if nc is None:
        nc = build_graph(cap)
        _graph_cache[cap] = nc
    res = run_bass_kernel_spmd(nc, in_maps, core_ids=list(range(NCORES)))
    energies = assemble_output(res.results, meta)
    return (species, energies)
